# revision 47
# baseline (speedup 1.0000x reference)
"""Trainium2 Bass kernel: 16-head RoPE attention block (B=4, T=2048, D=2048).

Sharding: tensor-parallel over heads. Each of the 8 cores owns 2 heads
(a 256-wide slice of the q/k/v projection output features). Per core:

  stage 1: q/k projections feature-major (stationary = W^T tiles, moving
           = x^T), RoPE on the vector engine; v projection emitted
           TRANSPOSED directly (stationary = x tile, moving = W^T) so no
           PE transposes are needed. Batch 0's q/k/v are copied straight
           into SBUF (no DRAM roundtrip); batches 1-3 stage through DRAM.
  stage 2: per (batch, head): scores computed TRANSPOSED (S^T[k,q] =
           kTile^T @ qT) so softmax->PV needs no P transpose; exp on the
           scalar engine (no max subtraction needed: scores ~ N(0,1));
           PV + a ones-row matmul (softmax denominators) accumulate on
           the PE interleaved with the score matmuls. Normalization is
           fully on-chip: reciprocal of the PSUM denominator row, then a
           K=1 matmul broadcasts it across partitions; the multiply is
           software-pipelined one query-chunk behind the matmul stream.
  stage 3: out-projection partial product (full D columns) feature-major.

Host sums the 8 partial outputs (the "all-reduce") and un-transposes.
All matmuls run in float32r (FP22 multiply, fp32 accumulate): full PE
throughput with ~1e-4 relative error.
"""

import math

import numpy as np

import concourse.bacc as bacc
import concourse.bass as bass
import concourse.mybir as mybir
import concourse.tile as tile
from concourse.bass_utils import run_bass_kernel_spmd

F32 = mybir.dt.float32
F32R = mybir.dt.float32r
BF16 = mybir.dt.bfloat16
EXP = mybir.ActivationFunctionType.Exp

# Problem shape (hardcoded; the harness calls kernel() with exactly these).
B = 4
T = 2048
D_MODEL = 2048
HEAD_DIM = 128
N_CORES = 8
ROPE_BASE = 10000.0

HPC = 2                      # heads per core
F_LOC = HPC * HEAD_DIM       # 256 local projection features per core
BT = B * T
TCH = 512                    # token chunk width (stages 1/3)
QCH = 512                    # query chunk width (stage 2)
SCALE = 1.0 / math.sqrt(HEAD_DIM)
S_LOOK = 4                   # score-matmul lookahead in the attention loop


def build_module(b=B, t=T, d_model=D_MODEL, n_cores=N_CORES):
    """Build the per-core Bass module. All cores run the same program on
    different data (pure SPMD, no collectives)."""
    bt = b * t
    dt_ = d_model // 128
    kt = t // 128
    tch = min(TCH, bt)
    qch = min(QCH, t)
    ntch = bt // tch
    nqc = t // qch
    cpb = t // tch           # stage-1/3 token chunks per batch

    nc = bacc.Bacc(None, target_bir_lowering=False)

    xT = nc.dram_tensor("xT", [d_model, bt], F32, kind="ExternalInput")
    wqT = nc.dram_tensor("wqT", [d_model, F_LOC], F32, kind="ExternalInput")
    wkT = nc.dram_tensor("wkT", [d_model, F_LOC], F32, kind="ExternalInput")
    wvT = nc.dram_tensor("wvT", [d_model, F_LOC], F32, kind="ExternalInput")
    woT = nc.dram_tensor("woT", [F_LOC, d_model], F32, kind="ExternalInput")
    cosT = nc.dram_tensor("cosT", [HEAD_DIM, t], F32, kind="ExternalInput")
    rsinT = nc.dram_tensor("rsinT", [HEAD_DIM, t], F32, kind="ExternalInput")
    onesc = nc.dram_tensor("onesc", [128, 1], F32, kind="ExternalInput")
    onesr = nc.dram_tensor("onesr", [1, 128], F32, kind="ExternalInput")
    # partial outputs in bf16: halves the output DMA and the host all-reduce
    # traffic; the 8-way host sum stays in fp32 (~0.2% relative error).
    outP = nc.dram_tensor("outP", [d_model, bt], BF16, kind="ExternalOutput")

    with tile.TileContext(nc) as tc:
        with (
            tc.tile_pool(name="const", bufs=1) as constp,
            tc.tile_pool(name="b0sb", bufs=1) as b0p,
            tc.tile_pool(name="s3w", bufs=1) as wopool,
            tc.tile_pool(name="dram", bufs=1, space="DRAM") as dram,
            tc.tile_pool(name="ps_mm", bufs=2, space="PSUM") as ps_mm,
            tc.tile_pool(name="ps_pv", bufs=2, space="PSUM") as ps_pv,
            tc.tile_pool(name="ps_dn", bufs=2, space="PSUM") as ps_dn,
        ):
            # ---- constants (gpsimd ring: keep the sync ring free for the
            # stage-1 weight/x loads that gate the first matmuls) ----
            cos_sb = constp.tile([128, t], F32)
            nc.gpsimd.dma_start(out=cos_sb, in_=cosT[:, :])
            rsin_sb = constp.tile([128, t], F32)
            nc.gpsimd.dma_start(out=rsin_sb, in_=rsinT[:, :])
            ones_sb = constp.tile([128, 1], F32R)
            nc.gpsimd.dma_start(out=ones_sb, in_=onesc[:, :].bitcast(F32R))
            onesr_sb = constp.tile([1, 128], F32R)
            nc.gpsimd.dma_start(out=onesr_sb, in_=onesr[:, :].bitcast(F32R))

            # batch-0 q/k/v live in SBUF end-to-end (written by stage 1,
            # read by stage 2); batches 1-3 stage through DRAM scratch.
            q_b0 = b0p.tile([128, HPC, t], F32R)
            k_b0 = b0p.tile([128, HPC, t], F32R)
            v_b0 = b0p.tile([128, kt, F_LOC], F32R)

            # out-projection weights preloaded on the scalar ring (sync
            # ring is saturated by the stage-1 x stream)
            wo_sb = wopool.tile([128, HPC, d_model], F32R, tag="wo")

            # ---- DRAM scratch for batches 1..b-1 (per batch so the
            # stage-2 loads become 3 large contiguous DMAs) ----
            q_scr = [
                dram.tile([128, HPC, t], F32, name=f"qs{bi}", tag=f"qs{bi}")
                for bi in range(b)
            ]
            k_scr = [
                dram.tile([128, HPC, t], F32, name=f"ks{bi}", tag=f"ks{bi}")
                for bi in range(b)
            ]
            v_scr = [
                dram.tile([t, F_LOC], F32, name=f"vs{bi}", tag=f"vs{bi}")
                for bi in range(b)
            ]

            # ================= stage 1: projections + rope =================
            with (
                tc.tile_pool(name="s1w", bufs=1) as wpool,
                tc.tile_pool(name="s1x", bufs=4) as xpool,
                tc.tile_pool(name="s1t", bufs=4) as tpool,
            ):
                w_sbs = []
                for wi, (wten, wname) in enumerate(
                    ((wqT, "wq"), (wkT, "wk"), (wvT, "wv"))
                ):
                    wsb = wpool.tile([128, dt_, F_LOC], F32R, tag=wname)
                    w_sbs.append(wsb)
                wq_src = wqT[:, :].rearrange("(dt p) f -> p dt f", p=128).bitcast(F32R)
                # wk on the scalar ring, wv on the gpsimd ring (parallel to
                # the sync-ring wq/x stream) so chunk 0's k/v projections
                # aren't serialized behind one slow ring; wo follows wk.
                nc.scalar.dma_start(
                    out=w_sbs[1],
                    in_=wkT[:, :].rearrange("(dt p) f -> p dt f", p=128).bitcast(F32R),
                )
                nc.gpsimd.dma_start(
                    out=w_sbs[2],
                    in_=wvT[:, :].rearrange("(dt p) f -> p dt f", p=128).bitcast(F32R),
                )
                nc.scalar.dma_start(
                    out=wo_sb,
                    in_=woT[:, :]
                    .rearrange("(ft p) d -> p ft d", p=128)
                    .bitcast(F32R),
                )

                hx = dt_ // 2    # x chunks split in halves of the d dim
                for tch_i in range(ntch):
                    bi = tch_i // cpb
                    off = (tch_i % cpb) * tch
                    lsl = slice(off, off + tch)
                    tsl = slice(tch_i * tch, (tch_i + 1) * tch)
                    xsrc = (
                        xT[:, tsl]
                        .rearrange("(dt p) tt -> p dt tt", p=128)
                        .bitcast(F32R)
                    )
                    xh = [
                        xpool.tile([128, hx, tch], F32R, tag="x", name=f"x{tch_i}_{hi}")
                        for hi in range(2)
                    ]
                    if tch_i == 0:
                        # interleave per-slice wq + x loads on the sync ring
                        # so the first matmul starts after ~one slice pair
                        for di in range(dt_):
                            nc.sync.dma_start(
                                out=w_sbs[0][:, di, :], in_=wq_src[:, di, :]
                            )
                            nc.sync.dma_start(
                                out=xh[di // hx][:, di % hx, :],
                                in_=xsrc[:, di, :],
                            )
                    else:
                        for hi in range(2):
                            nc.sync.dma_start(
                                out=xh[hi], in_=xsrc[:, hi * hx : (hi + 1) * hx, :]
                            )

                    def xsl(di, csl=slice(None)):
                        return xh[di // hx][:, di % hx, csl]

                    last_chunk = tch_i == ntch - 1
                    for pi in range(3):
                        if pi < 2:
                            # both heads' 128-feature groups accumulate into
                            # one [128, 2, tch] pair tile (2 PSUM banks). The
                            # final chunk borrows the pv/dn banks instead so
                            # its trailing rope reads never block stage 2's
                            # first score pairs (WAR on the mm2 slots).
                            if last_chunk:
                                pool_ = ps_pv if pi == 0 else ps_dn
                                pa = pool_.tile(
                                    [128, tch], F32,
                                    tag="pv" if pi == 0 else "dn",
                                )
                                pb = pool_.tile(
                                    [128, tch], F32,
                                    tag="pv" if pi == 0 else "dn",
                                )
                                ps2 = [pa, pb]
                            else:
                                ps2_t = ps_mm.tile(
                                    [128, HPC, tch], F32, tag="mm2"
                                )
                                ps2 = [ps2_t[:, 0, :], ps2_t[:, 1, :]]
                            for ft in range(HPC):
                                fsl = slice(ft * 128, (ft + 1) * 128)
                                for di in range(dt_):
                                    nc.tensor.matmul(
                                        ps2[ft],
                                        w_sbs[pi][:, di, fsl],
                                        xsl(di),
                                        start=(di == 0),
                                        stop=(di == dt_ - 1),
                                    )
                            # rope: out = in*cos + rot_half(in)*sin
                            ro = tpool.tile([128, HPC, tch], F32, tag="ro", bufs=2)
                            rt = tpool.tile([128, HPC, tch], F32, tag="rt", bufs=1)
                            for ft in range(HPC):
                                nc.vector.tensor_mul(
                                    ro[:, ft, :], ps2[ft], cos_sb[:, lsl]
                                )
                                nc.vector.tensor_mul(
                                    rt[0:64, ft, :],
                                    ps2[ft][64:128],
                                    rsin_sb[0:64, lsl],
                                )
                                nc.vector.tensor_mul(
                                    rt[64:128, ft, :],
                                    ps2[ft][0:64],
                                    rsin_sb[64:128, lsl],
                                )
                            nc.vector.tensor_add(ro, ro, rt)
                            if bi == 0:
                                dst = q_b0 if pi == 0 else k_b0
                                nc.scalar.copy(dst[:, :, lsl], ro)
                            else:
                                scr = q_scr if pi == 0 else k_scr
                                nc.gpsimd.dma_start(
                                    out=scr[bi][:, :, lsl], in_=ro
                                )
                        else:
                            # v emitted transposed: stationary = x token
                            # tile, moving = Wv^T -> psum [tokens, feats]
                            for ts in range(tch // 128):
                                csl = slice(ts * 128, (ts + 1) * 128)
                                ps = ps_pv.tile([128, F_LOC], F32, tag="pv")
                                for di in range(dt_):
                                    nc.tensor.matmul(
                                        ps,
                                        xsl(di, csl),
                                        w_sbs[2][:, di, :],
                                        start=(di == 0),
                                        stop=(di == dt_ - 1),
                                    )
                                blk = (tch_i % cpb) * (tch // 128) + ts
                                if bi == 0:
                                    nc.scalar.copy(v_b0[:, blk, :], ps)
                                else:
                                    vsb = tpool.tile(
                                        [128, F_LOC], F32, tag="vs", bufs=2
                                    )
                                    nc.scalar.copy(vsb, ps)
                                    nc.gpsimd.dma_start(
                                        out=v_scr[bi][
                                            blk * 128 : (blk + 1) * 128, :
                                        ],
                                        in_=vsb,
                                    )

            # ======== stage 2+3: attention + fused out-projection =========
            with (
                tc.tile_pool(name="s2in", bufs=2) as s2in,
                tc.tile_pool(name="s2", bufs=2) as s2pool,
                tc.tile_pool(name="s2e", bufs=4) as epool,
                tc.tile_pool(name="s3o", bufs=4) as s3pool,
            ):
                def load_qk(bi, h):
                    if bi == 0:
                        return (q_b0[:, h, :], k_b0[:, h, :])
                    q_sb = s2in.tile([128, t], F32R, tag="q")
                    nc.gpsimd.dma_start(
                        out=q_sb, in_=q_scr[bi][:, h, :].bitcast(F32R)
                    )
                    k_sb = s2in.tile([128, t], F32R, tag="k")
                    nc.gpsimd.dma_start(
                        out=k_sb, in_=k_scr[bi][:, h, :].bitcast(F32R)
                    )
                    return q_sb, k_sb

                def load_v(bi):
                    if bi == 0:
                        return v_b0
                    v_sb = s2in.tile([128, kt, F_LOC], F32R, tag="v")
                    nc.gpsimd.dma_start(
                        out=v_sb,
                        in_=v_scr[bi][:, :]
                        .rearrange("(tt p) f -> p tt f", p=128)
                        .bitcast(F32R),
                    )
                    return v_sb

                # Deferred normalization: at each qchunk's end the pv
                # evacuation + reciprocal run on the DVE; the PE broadcast
                # matmul + final multiply execute ~8us later (mid next
                # qchunk, or mid out-projection for the last qchunk) so the
                # PE never waits on the serial [1,qch] reciprocal.
                pending = [None]

                def do_norm():
                    if pending[0] is None:
                        return
                    an_p, h_p, au_p, rcp_p, qsl_p = pending[0]
                    pending[0] = None
                    # rec rotates through the dn tag (the fast reciprocal
                    # consumes each dn row well before its slot recycles);
                    # it must NOT share the pv tag: pv is still live here
                    rec = ps_dn.tile([128, qch], F32, tag="dn")
                    nc.tensor.matmul(
                        rec, onesr_sb, rcp_p, start=True, stop=True
                    )
                    nc.vector.tensor_mul(an_p[:, h_p, qsl_p], au_p, rec)

                cur_qk = load_qk(0, 0)
                cur_v = load_v(0)
                for bi in range(b):
                    v_sb = cur_v
                    # normalized attention for this batch, f32r, feeds the
                    # fused out-projection directly from SBUF
                    attn_n = s2pool.tile([128, HPC, t], F32R, tag="an", bufs=1)
                    for h in range(HPC):
                        q_sb, k_sb = cur_qk
                        if h + 1 < HPC:
                            cur_qk = load_qk(bi, h + 1)
                        elif bi + 1 < b:
                            cur_qk = load_qk(bi + 1, 0)
                            cur_v = load_v(bi + 1)

                        npair = kt // 2
                        for qc in range(nqc):
                            qsl = slice(qc * qch, (qc + 1) * qch)
                            e_tiles = [None] * npair

                            def emit_score_pair(p):
                                # two adjacent k-tiles share one 2-bank psum
                                # tile -> a single (amortized) exp activation
                                sps = ps_mm.tile([128, 2, qch], F32, tag="mm2")
                                for j in range(2):
                                    kti = 2 * p + j
                                    nc.tensor.matmul(
                                        sps[:, j, :],
                                        k_sb[:, kti * 128 : (kti + 1) * 128],
                                        q_sb[:, qsl],
                                        start=True,
                                        stop=True,
                                    )
                                e_sb = epool.tile(
                                    [128, 2, qch], F32R, tag="E", bufs=4
                                )
                                nc.scalar.activation(e_sb, sps, EXP, scale=SCALE)
                                e_tiles[p] = e_sb

                            for p in range(2):
                                emit_score_pair(p)
                            pv = ps_pv.tile([128, qch], F32, tag="pv")
                            dn = ps_dn.tile([1, qch], F32, tag="dn")
                            # denominator split: half the k-tiles reduce on
                            # the PE (ones-row passes); the rest fold on the
                            # DVE and join via one final matmul. The very
                            # first qchunk keeps its PE passes late so the dn
                            # bank's WAR on the last stage-1 rope never
                            # stalls the stage transition.
                            first_qc = bi == 0 and h == 0 and qc == 0
                            pe_ktis = (
                                (8, 10, 12, 14)
                                if first_qc
                                else (0, 2, 4, 6, 8, 10, 12, 14)
                            )
                            dve_ktis = [
                                k_
                                for k_ in range(kt)
                                if k_ not in pe_ktis
                            ]
                            acc = None
                            acc_r = None
                            n_add = 0
                            for kti in range(kt):
                                e_mv = e_tiles[kti // 2][:, kti % 2, :]
                                nc.tensor.matmul(
                                    pv,
                                    v_sb[:, kti, h * 128 : (h + 1) * 128],
                                    e_mv,
                                    start=(kti == 0),
                                    stop=(kti == kt - 1),
                                )
                                if kti in pe_ktis:
                                    nc.tensor.matmul(
                                        dn,
                                        ones_sb,
                                        e_mv,
                                        start=(kti == pe_ktis[0]),
                                        stop=False,
                                    )
                                elif kti in dve_ktis and kti >= dve_ktis[1]:
                                    # fold the next ready dve tile
                                    a_k = dve_ktis[n_add + 1]
                                    a_mv = e_tiles[a_k // 2][:, a_k % 2, :]
                                    if n_add == 0:
                                        f_k = dve_ktis[0]
                                        f_mv = e_tiles[f_k // 2][:, f_k % 2, :]
                                        acc = s2pool.tile(
                                            [128, qch], F32, tag="acc"
                                        )
                                        nc.vector.tensor_add(acc, f_mv, a_mv)
                                    elif n_add == len(dve_ktis) - 2:
                                        acc_r = s2pool.tile(
                                            [128, qch], F32R, tag="accr"
                                        )
                                        nc.vector.tensor_add(acc_r, acc, a_mv)
                                    else:
                                        nc.vector.tensor_add(acc, acc, a_mv)
                                    n_add += 1
                                if kti == 12:
                                    do_norm()
                                if kti % 2 == 1 and (kti + 4) < kt:
                                    emit_score_pair((kti + 4) // 2)
                            nc.tensor.matmul(
                                dn, ones_sb, acc_r, start=False, stop=True
                            )
                            au = s2pool.tile([128, qch], F32, tag="au")
                            hq = qch // 2
                            nc.scalar.copy(au[:, 0:hq], pv[:, 0:hq])
                            nc.vector.tensor_copy(au[:, hq:], pv[:, hq:])
                            rcp = s2pool.tile([1, qch], F32, tag="rcp")
                            nc.vector.reciprocal_approx_fast(out=rcp, in_=dn)
                            rcpr = s2pool.tile([1, qch], F32R, tag="rcpr")
                            nc.vector.tensor_copy(rcpr, rcp)
                            pending[0] = (attn_n, h, au, rcpr, qsl)

                    # ---- fused out-projection for this batch ----
                    for c4 in range(cpb):
                        off = c4 * tch
                        gsl = slice(bi * t + off, bi * t + off + tch)
                        for dp in range(dt_ // 2):
                            if dp == 4 and c4 == 2:
                                do_norm()
                            # the final chunk's tail quads borrow the pv/dn
                            # banks so the next batch's first score pairs
                            # never wait on a late mm2 evacuation
                            tail = c4 == cpb - 1 and dp >= 4
                            if tail:
                                ps_a = ps_pv.tile(
                                    [128, tch], F32, tag="pv", name=f"s3pv{dp}"
                                )
                                ps_b = ps_dn.tile(
                                    [128, tch], F32, tag="dn", name=f"s3dn{dp}"
                                )
                                ps_list = [ps_a, ps_b]
                            else:
                                ps_t = ps_mm.tile([128, 2, tch], F32, tag="mm2")
                                ps_list = [ps_t[:, 0, :], ps_t[:, 1, :]]
                            for j in range(2):
                                do = 2 * dp + j
                                for ft in range(HPC):
                                    nc.tensor.matmul(
                                        ps_list[j],
                                        wo_sb[:, ft, do * 128 : (do + 1) * 128],
                                        attn_n[:, ft, off : off + tch],
                                        start=(ft == 0),
                                        stop=(ft == HPC - 1),
                                    )
                            osb = s3pool.tile([128, 2, tch], BF16, tag="o")
                            if tail:
                                # separate psum tiles -> one copy each, on
                                # both engines in parallel
                                nc.vector.tensor_copy(osb[:, 0, :], ps_list[0])
                                nc.scalar.copy(osb[:, 1, :], ps_list[1])
                            elif dp % 2 == 0:
                                nc.vector.tensor_copy(osb, ps_t)
                            else:
                                nc.scalar.copy(osb, ps_t)
                            dst = (
                                outP[dp * 256 : (dp + 1) * 256, gsl]
                                .rearrange("(g p) tt -> p g tt", p=128)
                            )
                            # final chunk's writes split across both rings so
                            # the post-compute drain is halved
                            if bi == b - 1 and c4 == cpb - 1 and dp % 2 == 0:
                                nc.gpsimd.dma_start(out=dst, in_=osb)
                            else:
                                nc.sync.dma_start(out=dst, in_=osb)

    nc.finalize()
    return nc


_module_cache = {}


def _get_module(b, t, d_model, n_cores):
    key = (b, t, d_model, n_cores)
    if key not in _module_cache:
        _module_cache[key] = build_module(b, t, d_model, n_cores)
    return _module_cache[key]


def _host_tables(t):
    half = HEAD_DIM // 2
    theta = 1.0 / (
        np.float32(ROPE_BASE)
        ** (np.arange(half, dtype=np.float32) / np.float32(half))
    )
    freqs = np.arange(t, dtype=np.float32)[:, None] * theta[None, :]
    emb = np.concatenate([freqs, freqs], axis=-1)  # (t, 128)
    cosT = np.ascontiguousarray(np.cos(emb).T.astype(np.float32))
    sinT = np.sin(emb).T.astype(np.float32)
    rsinT = sinT.copy()
    rsinT[:half] = -sinT[:half]
    rsinT = np.ascontiguousarray(rsinT)
    return cosT, rsinT


def _run(x, Wq, Wk, Wv, Wo, trace=False):
    b_, t_, d_ = x.shape
    n_cores = (d_ // HEAD_DIM) // HPC
    nc = _get_module(b_, t_, d_, n_cores)

    xT = np.ascontiguousarray(x.reshape(b_ * t_, d_).T)
    cosT, rsinT = _host_tables(t_)
    onesc = np.ones((128, 1), dtype=np.float32)
    onesr = np.ones((1, 128), dtype=np.float32)

    in_maps = []
    for c in range(n_cores):
        fs = slice(c * F_LOC, (c + 1) * F_LOC)
        in_maps.append(
            {
                "xT": xT,
                "wqT": np.ascontiguousarray(Wq[fs, :].T),
                "wkT": np.ascontiguousarray(Wk[fs, :].T),
                "wvT": np.ascontiguousarray(Wv[fs, :].T),
                "woT": np.ascontiguousarray(Wo[:, fs].T),
                "cosT": cosT,
                "rsinT": rsinT,
                "onesc": onesc,
                "onesr": onesr,
            }
        )
    res = run_bass_kernel_spmd(
        nc, in_maps, core_ids=list(range(n_cores)), trace=trace
    )

    def bf16_to_f32(a):
        a = np.asarray(a)
        if a.dtype == np.float32:
            return a
        return (
            a.view(np.uint16).astype(np.uint32) << 16
        ).view(np.float32)

    acc = bf16_to_f32(res.results[0]["outP"]).copy()
    for c in range(1, n_cores):
        acc += bf16_to_f32(res.results[c]["outP"])
    out = np.ascontiguousarray(acc.T).reshape(b_, t_, d_)
    return out, res


def kernel(x, Wq, Wk, Wv, Wo):
    x = np.asarray(x, dtype=np.float32)
    Wq = np.asarray(Wq, dtype=np.float32)
    Wk = np.asarray(Wk, dtype=np.float32)
    Wv = np.asarray(Wv, dtype=np.float32)
    Wo = np.asarray(Wo, dtype=np.float32)
    out, _ = _run(x, Wq, Wk, Wv, Wo, trace=False)
    return out


# revision 52
# speedup vs baseline: 1.0172x; 1.0172x over previous
"""Trainium2 Bass kernel: 16-head RoPE attention block (B=4, T=2048, D=2048).

Sharding: tensor-parallel over heads. Each of the 8 cores owns 2 heads
(a 256-wide slice of the q/k/v projection output features). Per core:

  stage 1: q/k projections feature-major (stationary = W^T tiles, moving
           = x^T), RoPE on the vector engine; v projection emitted
           TRANSPOSED directly (stationary = x tile, moving = W^T) so no
           PE transposes are needed. Batch 0's q/k/v are copied straight
           into SBUF (no DRAM roundtrip); batches 1-3 stage through DRAM.
  stage 2: per (batch, head): scores computed TRANSPOSED (S^T[k,q] =
           kTile^T @ qT) so softmax->PV needs no P transpose; exp on the
           scalar engine (no max subtraction needed: scores ~ N(0,1));
           PV + a ones-row matmul (softmax denominators) accumulate on
           the PE interleaved with the score matmuls. Normalization is
           fully on-chip: reciprocal of the PSUM denominator row, then a
           K=1 matmul broadcasts it across partitions; the multiply is
           software-pipelined one query-chunk behind the matmul stream.
  stage 3: out-projection partial product (full D columns) feature-major.

Host sums the 8 partial outputs (the "all-reduce") and un-transposes.
All matmuls run in float32r (FP22 multiply, fp32 accumulate): full PE
throughput with ~1e-4 relative error.
"""

import math

import numpy as np

import concourse.bacc as bacc
import concourse.bass as bass
import concourse.mybir as mybir
import concourse.tile as tile
from concourse.bass_utils import run_bass_kernel_spmd

F32 = mybir.dt.float32
F32R = mybir.dt.float32r
BF16 = mybir.dt.bfloat16
EXP = mybir.ActivationFunctionType.Exp

# Problem shape (hardcoded; the harness calls kernel() with exactly these).
B = 4
T = 2048
D_MODEL = 2048
HEAD_DIM = 128
N_CORES = 8
ROPE_BASE = 10000.0

HPC = 2                      # heads per core
F_LOC = HPC * HEAD_DIM       # 256 local projection features per core
BT = B * T
TCH = 512                    # token chunk width (stages 1/3)
QCH = 512                    # query chunk width (stage 2)
SCALE = 1.0 / math.sqrt(HEAD_DIM)
S_LOOK = 4                   # score-matmul lookahead in the attention loop


def build_module(b=B, t=T, d_model=D_MODEL, n_cores=N_CORES):
    """Build the per-core Bass module. All cores run the same program on
    different data (pure SPMD, no collectives)."""
    bt = b * t
    dt_ = d_model // 128
    kt = t // 128
    tch = min(TCH, bt)
    qch = min(QCH, t)
    ntch = bt // tch
    nqc = t // qch
    cpb = t // tch           # stage-1/3 token chunks per batch

    nc = bacc.Bacc(None, target_bir_lowering=False)

    xT = nc.dram_tensor("xT", [d_model, bt], F32, kind="ExternalInput")
    wqT = nc.dram_tensor("wqT", [d_model, F_LOC], F32, kind="ExternalInput")
    wkT = nc.dram_tensor("wkT", [d_model, F_LOC], F32, kind="ExternalInput")
    wvT = nc.dram_tensor("wvT", [d_model, F_LOC], F32, kind="ExternalInput")
    woT = nc.dram_tensor("woT", [F_LOC, d_model], F32, kind="ExternalInput")
    cosT = nc.dram_tensor("cosT", [HEAD_DIM, t], F32, kind="ExternalInput")
    rsinT = nc.dram_tensor("rsinT", [HEAD_DIM, t], F32, kind="ExternalInput")
    onesc = nc.dram_tensor("onesc", [128, 1], F32, kind="ExternalInput")
    onesr = nc.dram_tensor("onesr", [1, 128], F32, kind="ExternalInput")
    # partial outputs in bf16: halves the output DMA and the host all-reduce
    # traffic; the 8-way host sum stays in fp32 (~0.2% relative error).
    outP = nc.dram_tensor("outP", [d_model, bt], BF16, kind="ExternalOutput")

    with tile.TileContext(nc) as tc:
        with (
            tc.tile_pool(name="const", bufs=1) as constp,
            tc.tile_pool(name="b0sb", bufs=1) as b0p,
            tc.tile_pool(name="s3w", bufs=1) as wopool,
            tc.tile_pool(name="dram", bufs=1, space="DRAM") as dram,
            tc.tile_pool(name="ps_mm", bufs=2, space="PSUM") as ps_mm,
            tc.tile_pool(name="ps_pv", bufs=2, space="PSUM") as ps_pv,
            tc.tile_pool(name="ps_dn", bufs=2, space="PSUM") as ps_dn,
        ):
            # ---- constants (gpsimd ring: keep the sync ring free for the
            # stage-1 weight/x loads that gate the first matmuls) ----
            cos_sb = constp.tile([128, t], F32)
            nc.gpsimd.dma_start(out=cos_sb, in_=cosT[:, :])
            rsin_sb = constp.tile([128, t], F32)
            nc.gpsimd.dma_start(out=rsin_sb, in_=rsinT[:, :])
            ones_sb = constp.tile([128, 1], F32R)
            nc.gpsimd.dma_start(out=ones_sb, in_=onesc[:, :].bitcast(F32R))
            onesr_sb = constp.tile([1, 128], F32R)
            nc.gpsimd.dma_start(out=onesr_sb, in_=onesr[:, :].bitcast(F32R))

            # batch-0 q/k/v live in SBUF end-to-end (written by stage 1,
            # read by stage 2); batches 1-3 stage through DRAM scratch.
            q_b0 = b0p.tile([128, HPC, t], F32R)
            k_b0 = b0p.tile([128, HPC, t], F32R)
            v_b0 = b0p.tile([128, kt, F_LOC], F32R)

            # out-projection weights preloaded on the scalar ring (sync
            # ring is saturated by the stage-1 x stream)
            wo_sb = wopool.tile([128, HPC, d_model], F32R, tag="wo")

            # ---- DRAM scratch for batches 1..b-1 (per batch so the
            # stage-2 loads become 3 large contiguous DMAs) ----
            q_scr = [
                dram.tile([128, HPC, t], F32, name=f"qs{bi}", tag=f"qs{bi}")
                for bi in range(b)
            ]
            k_scr = [
                dram.tile([128, HPC, t], F32, name=f"ks{bi}", tag=f"ks{bi}")
                for bi in range(b)
            ]
            v_scr = [
                dram.tile([t, F_LOC], F32, name=f"vs{bi}", tag=f"vs{bi}")
                for bi in range(b)
            ]

            # ================= stage 1: projections + rope =================
            with (
                tc.tile_pool(name="s1w", bufs=1) as wpool,
                tc.tile_pool(name="s1x", bufs=4) as xpool,
                tc.tile_pool(name="s1t", bufs=4) as tpool,
            ):
                w_sbs = []
                for wi, (wten, wname) in enumerate(
                    ((wqT, "wq"), (wkT, "wk"), (wvT, "wv"))
                ):
                    wsb = wpool.tile([128, dt_, F_LOC], F32R, tag=wname)
                    w_sbs.append(wsb)
                wq_src = wqT[:, :].rearrange("(dt p) f -> p dt f", p=128).bitcast(F32R)
                # wk on the scalar ring, wv on the gpsimd ring (parallel to
                # the sync-ring wq/x stream) so chunk 0's k/v projections
                # aren't serialized behind one slow ring; wo follows wk.
                nc.scalar.dma_start(
                    out=w_sbs[1],
                    in_=wkT[:, :].rearrange("(dt p) f -> p dt f", p=128).bitcast(F32R),
                )
                nc.gpsimd.dma_start(
                    out=w_sbs[2],
                    in_=wvT[:, :].rearrange("(dt p) f -> p dt f", p=128).bitcast(F32R),
                )
                nc.scalar.dma_start(
                    out=wo_sb,
                    in_=woT[:, :]
                    .rearrange("(ft p) d -> p ft d", p=128)
                    .bitcast(F32R),
                )

                hx = dt_ // 2    # x chunks split in halves of the d dim
                for tch_i in range(ntch):
                    bi = tch_i // cpb
                    off = (tch_i % cpb) * tch
                    lsl = slice(off, off + tch)
                    tsl = slice(tch_i * tch, (tch_i + 1) * tch)
                    xsrc = (
                        xT[:, tsl]
                        .rearrange("(dt p) tt -> p dt tt", p=128)
                        .bitcast(F32R)
                    )
                    xh = [
                        xpool.tile([128, hx, tch], F32R, tag="x", name=f"x{tch_i}_{hi}")
                        for hi in range(2)
                    ]
                    if tch_i == 0:
                        # interleave wq + x loads on the sync ring: per-di
                        # for the first 4 slices (earliest first matmul),
                        # then 4-di blocks — each dma_start costs ~0.9us of
                        # SP descriptor-gen time, so 32 tiny issues would
                        # starve the ring mid-startup
                        for di in range(4):
                            nc.sync.dma_start(
                                out=w_sbs[0][:, di, :], in_=wq_src[:, di, :]
                            )
                            nc.sync.dma_start(
                                out=xh[0][:, di, :],
                                in_=xsrc[:, di, :],
                            )
                        for d0 in range(4, dt_, 4):
                            dsl = slice(d0, d0 + 4)
                            nc.sync.dma_start(
                                out=w_sbs[0][:, dsl, :], in_=wq_src[:, dsl, :]
                            )
                            if d0 // hx == (d0 + 3) // hx:
                                nc.sync.dma_start(
                                    out=xh[d0 // hx][:, d0 % hx : d0 % hx + 4, :],
                                    in_=xsrc[:, dsl, :],
                                )
                            else:
                                for di in range(d0, d0 + 4):
                                    nc.sync.dma_start(
                                        out=xh[di // hx][:, di % hx, :],
                                        in_=xsrc[:, di, :],
                                    )
                    else:
                        for hi in range(2):
                            nc.sync.dma_start(
                                out=xh[hi], in_=xsrc[:, hi * hx : (hi + 1) * hx, :]
                            )

                    def xsl(di, csl=slice(None)):
                        return xh[di // hx][:, di % hx, csl]

                    last_chunk = tch_i == ntch - 1
                    for pi in range(3):
                        if pi < 2:
                            # both heads' 128-feature groups accumulate into
                            # one [128, 2, tch] pair tile (2 PSUM banks). The
                            # final chunk borrows the pv/dn banks instead so
                            # its trailing rope reads never block stage 2's
                            # first score pairs (WAR on the mm2 slots).
                            if last_chunk:
                                pool_ = ps_pv if pi == 0 else ps_dn
                                pa = pool_.tile(
                                    [128, tch], F32,
                                    tag="pv" if pi == 0 else "dn",
                                )
                                pb = pool_.tile(
                                    [128, tch], F32,
                                    tag="pv" if pi == 0 else "dn",
                                )
                                ps2 = [pa, pb]
                            else:
                                ps2_t = ps_mm.tile(
                                    [128, HPC, tch], F32, tag="mm2"
                                )
                                ps2 = [ps2_t[:, 0, :], ps2_t[:, 1, :]]
                            for ft in range(HPC):
                                fsl = slice(ft * 128, (ft + 1) * 128)
                                for di in range(dt_):
                                    nc.tensor.matmul(
                                        ps2[ft],
                                        w_sbs[pi][:, di, fsl],
                                        xsl(di),
                                        start=(di == 0),
                                        stop=(di == dt_ - 1),
                                    )
                            # rope: out = in*cos + rot_half(in)*sin
                            ro = tpool.tile([128, HPC, tch], F32, tag="ro", bufs=2)
                            rt = tpool.tile([128, HPC, tch], F32, tag="rt", bufs=1)
                            for ft in range(HPC):
                                nc.vector.tensor_mul(
                                    ro[:, ft, :], ps2[ft], cos_sb[:, lsl]
                                )
                                nc.vector.tensor_mul(
                                    rt[0:64, ft, :],
                                    ps2[ft][64:128],
                                    rsin_sb[0:64, lsl],
                                )
                                nc.vector.tensor_mul(
                                    rt[64:128, ft, :],
                                    ps2[ft][0:64],
                                    rsin_sb[64:128, lsl],
                                )
                            nc.vector.tensor_add(ro, ro, rt)
                            if bi == 0:
                                dst = q_b0 if pi == 0 else k_b0
                                nc.scalar.copy(dst[:, :, lsl], ro)
                            else:
                                scr = q_scr if pi == 0 else k_scr
                                nc.gpsimd.dma_start(
                                    out=scr[bi][:, :, lsl], in_=ro
                                )
                        else:
                            # v emitted transposed: stationary = x token
                            # tile, moving = Wv^T -> psum [tokens, feats]
                            for ts in range(tch // 128):
                                csl = slice(ts * 128, (ts + 1) * 128)
                                ps = ps_pv.tile([128, F_LOC], F32, tag="pv")
                                for di in range(dt_):
                                    nc.tensor.matmul(
                                        ps,
                                        xsl(di, csl),
                                        w_sbs[2][:, di, :],
                                        start=(di == 0),
                                        stop=(di == dt_ - 1),
                                    )
                                blk = (tch_i % cpb) * (tch // 128) + ts
                                if bi == 0:
                                    nc.scalar.copy(v_b0[:, blk, :], ps)
                                else:
                                    vsb = tpool.tile(
                                        [128, F_LOC], F32, tag="vs", bufs=2
                                    )
                                    nc.scalar.copy(vsb, ps)
                                    nc.gpsimd.dma_start(
                                        out=v_scr[bi][
                                            blk * 128 : (blk + 1) * 128, :
                                        ],
                                        in_=vsb,
                                    )

            # ======== stage 2+3: attention + fused out-projection =========
            with (
                tc.tile_pool(name="s2in", bufs=2) as s2in,
                tc.tile_pool(name="s2", bufs=2) as s2pool,
                tc.tile_pool(name="s2e", bufs=4) as epool,
                tc.tile_pool(name="s3o", bufs=4) as s3pool,
            ):
                def load_qk(bi, h):
                    if bi == 0:
                        return (q_b0[:, h, :], k_b0[:, h, :])
                    q_sb = s2in.tile([128, t], F32R, tag="q")
                    nc.gpsimd.dma_start(
                        out=q_sb, in_=q_scr[bi][:, h, :].bitcast(F32R)
                    )
                    k_sb = s2in.tile([128, t], F32R, tag="k")
                    nc.gpsimd.dma_start(
                        out=k_sb, in_=k_scr[bi][:, h, :].bitcast(F32R)
                    )
                    return q_sb, k_sb

                def load_v(bi):
                    if bi == 0:
                        return v_b0
                    v_sb = s2in.tile([128, kt, F_LOC], F32R, tag="v")
                    nc.gpsimd.dma_start(
                        out=v_sb,
                        in_=v_scr[bi][:, :]
                        .rearrange("(tt p) f -> p tt f", p=128)
                        .bitcast(F32R),
                    )
                    return v_sb

                # Deferred normalization: at each qchunk's end the pv
                # evacuation + reciprocal run on the DVE; the PE broadcast
                # matmul + final multiply execute ~8us later (mid next
                # qchunk, or mid out-projection for the last qchunk) so the
                # PE never waits on the serial [1,qch] reciprocal.
                pending = [None]

                def do_norm():
                    if pending[0] is None:
                        return
                    an_p, h_p, au_p, rcp_p, qsl_p = pending[0]
                    pending[0] = None
                    # rec rotates through the dn tag (the fast reciprocal
                    # consumes each dn row well before its slot recycles);
                    # it must NOT share the pv tag: pv is still live here
                    rec = ps_dn.tile([128, qch], F32, tag="dn")
                    nc.tensor.matmul(
                        rec, onesr_sb, rcp_p, start=True, stop=True
                    )
                    nc.vector.tensor_mul(an_p[:, h_p, qsl_p], au_p, rec)

                cur_qk = load_qk(0, 0)
                cur_v = load_v(0)
                for bi in range(b):
                    v_sb = cur_v
                    # normalized attention for this batch, f32r, feeds the
                    # fused out-projection directly from SBUF
                    attn_n = s2pool.tile([128, HPC, t], F32R, tag="an", bufs=1)
                    for h in range(HPC):
                        q_sb, k_sb = cur_qk
                        if h + 1 < HPC:
                            cur_qk = load_qk(bi, h + 1)
                        elif bi + 1 < b:
                            cur_qk = load_qk(bi + 1, 0)
                            cur_v = load_v(bi + 1)

                        npair = kt // 2
                        for qc in range(nqc):
                            qsl = slice(qc * qch, (qc + 1) * qch)
                            e_tiles = [None] * npair

                            def emit_score_pair(p):
                                # two adjacent k-tiles share one 2-bank psum
                                # tile -> a single (amortized) exp activation
                                sps = ps_mm.tile([128, 2, qch], F32, tag="mm2")
                                for j in range(2):
                                    kti = 2 * p + j
                                    nc.tensor.matmul(
                                        sps[:, j, :],
                                        k_sb[:, kti * 128 : (kti + 1) * 128],
                                        q_sb[:, qsl],
                                        start=True,
                                        stop=True,
                                    )
                                e_sb = epool.tile(
                                    [128, 2, qch], F32R, tag="E", bufs=4
                                )
                                nc.scalar.activation(e_sb, sps, EXP, scale=SCALE)
                                e_tiles[p] = e_sb

                            for p in range(2):
                                emit_score_pair(p)
                            pv = ps_pv.tile([128, qch], F32, tag="pv")
                            dn = ps_dn.tile([1, qch], F32, tag="dn")
                            # denominator split: even k-tiles reduce on the
                            # PE (8 ones-row passes); odd k-tiles fold on the
                            # DVE (7 adds) and join via one final matmul.
                            acc = None
                            acc_r = None
                            for kti in range(kt):
                                e_mv = e_tiles[kti // 2][:, kti % 2, :]
                                nc.tensor.matmul(
                                    pv,
                                    v_sb[:, kti, h * 128 : (h + 1) * 128],
                                    e_mv,
                                    start=(kti == 0),
                                    stop=(kti == kt - 1),
                                )
                                if kti % 2 == 0:
                                    nc.tensor.matmul(
                                        dn,
                                        ones_sb,
                                        e_mv,
                                        start=(kti == 0),
                                        stop=False,
                                    )
                                else:
                                    p = kti // 2
                                    if p == 1:
                                        acc = s2pool.tile(
                                            [128, qch], F32, tag="acc"
                                        )
                                        nc.vector.tensor_add(
                                            acc,
                                            e_tiles[0][:, 1, :],
                                            e_tiles[1][:, 1, :],
                                        )
                                    elif p == kt // 2 - 1:
                                        acc_r = s2pool.tile(
                                            [128, qch], F32R, tag="accr"
                                        )
                                        nc.vector.tensor_add(
                                            acc_r, acc, e_mv
                                        )
                                    elif p > 1:
                                        nc.vector.tensor_add(acc, acc, e_mv)
                                if kti == 12:
                                    do_norm()
                                if kti % 2 == 1 and (kti + 4) < kt:
                                    emit_score_pair((kti + 4) // 2)
                            nc.tensor.matmul(
                                dn, ones_sb, acc_r, start=False, stop=True
                            )
                            au = s2pool.tile([128, qch], F32, tag="au")
                            nc.scalar.copy(au, pv)
                            rcp = s2pool.tile([1, qch], F32, tag="rcp")
                            nc.vector.reciprocal_approx_fast(out=rcp, in_=dn)
                            rcpr = s2pool.tile([1, qch], F32R, tag="rcpr")
                            nc.vector.tensor_copy(rcpr, rcp)
                            pending[0] = (attn_n, h, au, rcpr, qsl)

                    # ---- fused out-projection for this batch ----
                    for c4 in range(cpb):
                        off = c4 * tch
                        gsl = slice(bi * t + off, bi * t + off + tch)
                        for dp in range(dt_ // 2):
                            if dp == 4 and c4 == 2:
                                do_norm()
                            ps = ps_mm.tile([128, 2, tch], F32, tag="mm2")
                            for j in range(2):
                                do = 2 * dp + j
                                for ft in range(HPC):
                                    nc.tensor.matmul(
                                        ps[:, j, :],
                                        wo_sb[:, ft, do * 128 : (do + 1) * 128],
                                        attn_n[:, ft, off : off + tch],
                                        start=(ft == 0),
                                        stop=(ft == HPC - 1),
                                    )
                            osb = s3pool.tile([128, 2, tch], BF16, tag="o")
                            if dp % 2 == 0:
                                nc.vector.tensor_copy(osb, ps)
                            else:
                                nc.scalar.copy(osb, ps)
                            dst = (
                                outP[dp * 256 : (dp + 1) * 256, gsl]
                                .rearrange("(g p) tt -> p g tt", p=128)
                            )
                            # final chunk's writes split across both rings so
                            # the post-compute drain is halved
                            if bi == b - 1 and c4 == cpb - 1 and dp % 2 == 0:
                                nc.gpsimd.dma_start(out=dst, in_=osb)
                            else:
                                nc.sync.dma_start(out=dst, in_=osb)

    nc.finalize()
    return nc


_module_cache = {}


def _get_module(b, t, d_model, n_cores):
    key = (b, t, d_model, n_cores)
    if key not in _module_cache:
        _module_cache[key] = build_module(b, t, d_model, n_cores)
    return _module_cache[key]


def _host_tables(t):
    half = HEAD_DIM // 2
    theta = 1.0 / (
        np.float32(ROPE_BASE)
        ** (np.arange(half, dtype=np.float32) / np.float32(half))
    )
    freqs = np.arange(t, dtype=np.float32)[:, None] * theta[None, :]
    emb = np.concatenate([freqs, freqs], axis=-1)  # (t, 128)
    cosT = np.ascontiguousarray(np.cos(emb).T.astype(np.float32))
    sinT = np.sin(emb).T.astype(np.float32)
    rsinT = sinT.copy()
    rsinT[:half] = -sinT[:half]
    rsinT = np.ascontiguousarray(rsinT)
    return cosT, rsinT


def _run(x, Wq, Wk, Wv, Wo, trace=False):
    b_, t_, d_ = x.shape
    n_cores = (d_ // HEAD_DIM) // HPC
    nc = _get_module(b_, t_, d_, n_cores)

    xT = np.ascontiguousarray(x.reshape(b_ * t_, d_).T)
    cosT, rsinT = _host_tables(t_)
    onesc = np.ones((128, 1), dtype=np.float32)
    onesr = np.ones((1, 128), dtype=np.float32)

    in_maps = []
    for c in range(n_cores):
        fs = slice(c * F_LOC, (c + 1) * F_LOC)
        in_maps.append(
            {
                "xT": xT,
                "wqT": np.ascontiguousarray(Wq[fs, :].T),
                "wkT": np.ascontiguousarray(Wk[fs, :].T),
                "wvT": np.ascontiguousarray(Wv[fs, :].T),
                "woT": np.ascontiguousarray(Wo[:, fs].T),
                "cosT": cosT,
                "rsinT": rsinT,
                "onesc": onesc,
                "onesr": onesr,
            }
        )
    res = run_bass_kernel_spmd(
        nc, in_maps, core_ids=list(range(n_cores)), trace=trace
    )

    def bf16_to_f32(a):
        a = np.asarray(a)
        if a.dtype == np.float32:
            return a
        return (
            a.view(np.uint16).astype(np.uint32) << 16
        ).view(np.float32)

    acc = bf16_to_f32(res.results[0]["outP"]).copy()
    for c in range(1, n_cores):
        acc += bf16_to_f32(res.results[c]["outP"])
    out = np.ascontiguousarray(acc.T).reshape(b_, t_, d_)
    return out, res


def kernel(x, Wq, Wk, Wv, Wo):
    x = np.asarray(x, dtype=np.float32)
    Wq = np.asarray(Wq, dtype=np.float32)
    Wk = np.asarray(Wk, dtype=np.float32)
    Wv = np.asarray(Wv, dtype=np.float32)
    Wo = np.asarray(Wo, dtype=np.float32)
    out, _ = _run(x, Wq, Wk, Wv, Wo, trace=False)
    return out
